# revision 1
# baseline (speedup 1.0000x reference)
"""HCLT probabilistic-circuit kernel for 8 Trainium2 NeuronCores.

Math: the reference collapses algebraically. With
  lp0 + lp1 summed in log space, exp'd, mixed by w_sum, then logsumexp'd,
the whole network is
  out[b] = log( sum_{k,m} w_sum[k] * W0[k,m,x0_b] * W1[k,m,x1_b] )
        = log( A[x0_b, x1_b] ),   A = sum_k w_k * W0[k].T @ W1[k]  (shape [C, C])

Distribution: shard the latent axis k (256) across 8 cores (32 each). Each core
reads only its W shard (134MB/8/2 in bf16 = 8.4MB), computes the partial
A_c = sum_{k in shard} w_k W0[k].T @ W1[k] via PSUM-accumulated matmuls, then
gathers its partial A_c at all 1024 (x0_b, x1_b) positions on-device
(one-hot matmul row-gather + fused mask-dot column-gather). The host sums the
8 partial gathered vectors (the unshard of the k-sharded reduction) and takes
the log.
"""

import sys

import numpy as np

sys.path.insert(0, "/opt/trn_rl_repo")

import ml_dtypes

B, V, M, C = 1024, 2, 256, 256
NCORES = 8
KSH = M // NCORES          # k per core = 32
KM = KSH * M               # flattened contraction rows per core = 8192
NCHUNK = KM // 128         # 64 matmul chunks of 128 rows
NBT = B // 128             # 8 batch tiles

_cache = {}


def _build_program():
    import concourse.bacc as bacc
    import concourse.mybir as mybir
    from concourse.tile import TileContext

    f32 = mybir.dt.float32
    bf16 = mybir.dt.bfloat16

    nc = bacc.Bacc("TRN2", target_bir_lowering=False)

    x0w = nc.dram_tensor("x0w", [128, NCHUNK * C], bf16, kind="ExternalInput")
    x1w = nc.dram_tensor("x1w", [128, NCHUNK * C], bf16, kind="ExternalInput")
    # aux packs f32 [wsc (64) | iota (256) | x1t (8)] per partition
    aux = nc.dram_tensor("aux", [128, NCHUNK + C + NBT], f32, kind="ExternalInput")
    oh0t = nc.dram_tensor("oh0t", [2 * 128, B], bf16, kind="ExternalInput")
    gout = nc.dram_tensor("gout", [128, NBT], f32, kind="ExternalOutput")

    NPIECE = 8
    PW = NCHUNK * C // NPIECE  # 2048 columns per DMA piece

    with TileContext(nc) as tc:
        with (
            tc.tile_pool(name="wp", bufs=1) as wp,
            tc.tile_pool(name="sp", bufs=3) as sp,
            tc.tile_pool(name="rp", bufs=4, space="PSUM") as rp,
            tc.tile_pool(name="apool", bufs=1, space="PSUM") as apool,
        ):
            x0sb = wp.tile([128, NCHUNK * C], bf16, name="x0sb")
            x1sb = wp.tile([128, NCHUNK * C], bf16, name="x1sb")
            x0s = wp.tile([128, NCHUNK * C], bf16, name="x0s")
            auxsb = wp.tile([128, NCHUNK + C + NBT], f32, name="auxsb")
            oh0sb = wp.tile([128, 2 * B], bf16, name="oh0sb")
            oh1sb = wp.tile([128, NBT * C], f32, name="oh1sb")
            gsb = wp.tile([128, NBT], f32, name="gsb")

            nc.sync.dma_start(out=auxsb[:], in_=aux[:])
            wscsb = auxsb[:, 0:NCHUNK]
            iotasb = auxsb[:, NCHUNK : NCHUNK + C]
            x1tsb = auxsb[:, NCHUNK + C : NCHUNK + C + NBT]

            # interleave the W-shard pieces so compute can chase the DMAs
            for p in range(NPIECE):
                sl = slice(p * PW, (p + 1) * PW)
                nc.sync.dma_start(out=x0sb[:, sl], in_=x0w[:, sl])
                nc.sync.dma_start(out=x1sb[:, sl], in_=x1w[:, sl])
            nc.sync.dma_start(out=oh0sb[:, 0:B], in_=oh0t[0:128, :])
            nc.sync.dma_start(out=oh0sb[:, B : 2 * B], in_=oh0t[128:256, :])

            # scale W0 chunks by their (uniform-per-chunk) w_sum factor
            for j in range(NCHUNK):
                sl = slice(j * C, (j + 1) * C)
                nc.vector.tensor_scalar(
                    out=x0s[:, sl],
                    in0=x0sb[:, sl],
                    scalar1=wscsb[:, j : j + 1],
                    scalar2=None,
                    op0=mybir.AluOpType.mult,
                )

            # build the 8 per-batch-tile x1 one-hot masks (needed only at
            # the gather stage; placed after the scales so the first scale
            # op -- which gates the first matmul -- issues as early as
            # possible on the in-order DVE queue)
            for i in range(NBT):
                nc.vector.tensor_scalar(
                    out=oh1sb[:, i * C : (i + 1) * C],
                    in0=iotasb,
                    scalar1=x1tsb[:, i : i + 1],
                    scalar2=None,
                    op0=mybir.AluOpType.is_equal,
                )

            # partial A = sum over 64 chunks of x0s_chunk.T @ x1_chunk
            a_ps = []
            for h in range(2):
                ah = apool.tile([128, C], f32, name=f"a{h}")
                a_ps.append(ah)
            # per DMA piece, run each PSUM half as a contiguous 8-MM
            # burst so LDWEIGHTS overlaps within a same-bank run
            CPP = NCHUNK // NPIECE
            for p in range(NPIECE):
                for h in range(2):
                    for j in range(p * CPP, (p + 1) * CPP):
                        nc.tensor.matmul(
                            a_ps[h],
                            lhsT=x0s[:, j * C + h * 128 : j * C + h * 128 + 128],
                            rhs=x1sb[:, j * C : (j + 1) * C],
                            start=(j == 0),
                            stop=(j == NCHUNK - 1),
                        )

            a_sb = []
            for h in range(2):
                ash = wp.tile([128, C], bf16, name=f"ash{h}")
                nc.vector.tensor_copy(ash, a_ps[h])
                a_sb.append(ash)

            # gather: R[b,:] = A[x0_b,:] via one-hot matmul, then dot with
            # the x1 one-hot row mask (built on-device) and reduce.
            for i in range(NBT):
                r_ps = rp.tile([128, C], mybir.dt.float32, name="r_ps")
                nc.tensor.matmul(
                    r_ps,
                    lhsT=oh0sb[:, i * 128 : (i + 1) * 128],
                    rhs=a_sb[0],
                    start=True,
                    stop=False,
                )
                nc.tensor.matmul(
                    r_ps,
                    lhsT=oh0sb[:, B + i * 128 : B + (i + 1) * 128],
                    rhs=a_sb[1],
                    start=False,
                    stop=True,
                )
                masked = sp.tile([128, C], f32, name="masked")
                nc.vector.tensor_tensor(
                    out=masked,
                    in0=r_ps,
                    in1=oh1sb[:, i * C : (i + 1) * C],
                    op=mybir.AluOpType.mult,
                )
                nc.vector.tensor_reduce(
                    out=gsb[:, i : i + 1],
                    in_=masked,
                    axis=mybir.AxisListType.X,
                    op=mybir.AluOpType.add,
                )

            nc.sync.dma_start(out=gout[:], in_=gsb[:])

    nc.compile()
    return nc


def _prep_inputs(x, W, w_sum):
    bf16 = ml_dtypes.bfloat16
    x = np.asarray(x)
    W = np.asarray(W, dtype=np.float32)
    w_sum = np.asarray(w_sum, dtype=np.float32)

    oh0t = np.zeros((C, B), dtype=bf16)
    oh0t[x[:, 0].astype(np.int64), np.arange(B)] = 1
    iotaf = np.broadcast_to(np.arange(C, dtype=np.float32)[None, :], (128, C))
    x1t = x[:, 1].astype(np.float32).reshape(NBT, 128).T

    in_maps = []
    for c in range(NCORES):
        k0 = c * KSH
        w0 = W[0, k0 : k0 + KSH].reshape(KM, C).astype(bf16)
        w1 = W[1, k0 : k0 + KSH].reshape(KM, C).astype(bf16)
        x0wc = np.ascontiguousarray(
            w0.reshape(NCHUNK, 128, C).transpose(1, 0, 2).reshape(128, NCHUNK * C)
        )
        x1wc = np.ascontiguousarray(
            w1.reshape(NCHUNK, 128, C).transpose(1, 0, 2).reshape(128, NCHUNK * C)
        )
        wsc = np.broadcast_to(
            np.repeat(w_sum[k0 : k0 + KSH], M // 128)[None, :], (128, NCHUNK)
        )
        auxc = np.ascontiguousarray(
            np.concatenate([wsc, iotaf, x1t], axis=1).astype(np.float32)
        )
        in_maps.append({"x0w": x0wc, "x1w": x1wc, "aux": auxc, "oh0t": oh0t})
    return in_maps


def _run(in_maps, **kwargs):
    from concourse.bass_utils import run_bass_kernel_spmd

    if "nc" not in _cache:
        _cache["nc"] = _build_program()
    return run_bass_kernel_spmd(
        _cache["nc"], in_maps, core_ids=list(range(NCORES)), **kwargs
    )


def kernel(x, W, w_sum):
    in_maps = _prep_inputs(x, W, w_sum)
    res = _run(in_maps)
    g = np.zeros((128, NBT), dtype=np.float64)
    for r in res.results:
        g += r["gout"].astype(np.float64)
    vals = g.T.reshape(B)  # b = tile*128 + partition
    return np.log(vals).astype(np.float32)



# revision 2
# speedup vs baseline: 1.5115x; 1.5115x over previous
"""HCLT probabilistic-circuit kernel for 8 Trainium2 NeuronCores.

Math: the reference collapses algebraically. With
  lp0 + lp1 summed in log space, exp'd, mixed by w_sum, then logsumexp'd,
the whole network is
  out[b] = log( sum_{k,m} w_sum[k] * W0[k,m,x0_b] * W1[k,m,x1_b] )
        = log( A[x0_b, x1_b] ),   A = sum_k w_k * W0[k].T @ W1[k]  (shape [C, C])

Distribution: shard the latent axis k (256) across 8 cores (32 each). Each core
streams its W shard in fp8e4 (sqrt(w_sum) folded into both factors on the host,
per-tensor scaled into fp8 range) and computes the partial
A_c = sum_{k in shard} w0q[k].T @ w1q[k] with DoubleRow fp8 matmuls (2
contraction chunks per instruction). The [256, 256] partial table is DMA'd out
in bf16; the host sums the 8 partials (the unshard of the k-sharded reduction),
applies the inverse scale, gathers the 1024 (x0_b, x1_b) entries and takes the
log.
"""

import sys

import numpy as np

sys.path.insert(0, "/opt/trn_rl_repo")

import ml_dtypes

B, V, M, C = 1024, 2, 256, 256
NCORES = 8
KSH = M // NCORES          # k per core = 32
KM = KSH * M               # flattened contraction rows per core = 8192
NCHUNK = KM // 128         # 64 matmul chunks of 128 rows
# W DMA pieces, in chunks: small first (start compute early), big later
PIECES = [4, 4, 8, 8, 16, 16, 8]

_cache = {}


def _build_program():
    import concourse.bacc as bacc
    import concourse.mybir as mybir
    from concourse.tile import TileContext

    f8 = mybir.dt.float8e4
    bf16 = mybir.dt.bfloat16
    f32 = mybir.dt.float32

    nc = bacc.Bacc("TRN2", target_bir_lowering=False)

    x0w = nc.dram_tensor("x0w", [128, NCHUNK * C], f8, kind="ExternalInput")
    x1w = nc.dram_tensor("x1w", [128, NCHUNK * C], f8, kind="ExternalInput")
    aout = nc.dram_tensor("aout", [128, 2 * C], bf16, kind="ExternalOutput")

    with TileContext(nc) as tc:
        with (
            tc.tile_pool(name="wp", bufs=1) as wp,
            tc.tile_pool(name="apool", bufs=1, space="PSUM") as apool,
        ):
            x0sb = wp.tile([128, NCHUNK, C], f8, name="x0sb")
            x1sb = wp.tile([128, NCHUNK, C], f8, name="x1sb")
            asb = wp.tile([128, 2, C], bf16, name="asb")
            a_ps = apool.tile([128, 2, C], f32, name="a_ps")

            j0 = 0
            for p in PIECES:
                sl = slice(j0 * C, (j0 + p) * C)
                nc.sync.dma_start(out=x0sb[:, j0 : j0 + p, :], in_=x0w[:, sl])
                nc.sync.dma_start(out=x1sb[:, j0 : j0 + p, :], in_=x1w[:, sl])
                j0 += p

            # partial A = sum over 64 chunks of x0q_chunk.T @ x1q_chunk,
            # two chunks per DoubleRow fp8 matmul
            for j in range(0, NCHUNK, 2):
                for h in range(2):
                    nc.tensor.matmul(
                        a_ps[:, h, :],
                        lhsT=x0sb[:, j : j + 2, h * 128 : h * 128 + 128],
                        rhs=x1sb[:, j : j + 2, :],
                        start=(j == 0),
                        stop=(j == NCHUNK - 2),
                        perf_mode=mybir.MatmulPerfMode.DoubleRow,
                    )

            nc.vector.tensor_copy(asb, a_ps)
            nc.sync.dma_start(out=aout[:], in_=asb[:])

    nc.compile()
    return nc


def _prep_inputs(x, W, w_sum):
    f8 = ml_dtypes.float8_e4m3
    W = np.asarray(W, dtype=np.float32)
    w_sum = np.asarray(w_sum, dtype=np.float32)

    sq = np.sqrt(w_sum)[:, None, None]
    w0 = W[0] * sq                      # [M(k), M(m), C]
    w1 = W[1] * sq
    s0 = 224.0 / float(w0.max())
    s1 = 224.0 / float(w1.max())
    q0 = (w0 * s0).astype(f8)
    q1 = (w1 * s1).astype(f8)

    in_maps = []
    for c in range(NCORES):
        k0 = c * KSH
        w0c = q0[k0 : k0 + KSH].reshape(KM, C)
        w1c = q1[k0 : k0 + KSH].reshape(KM, C)
        x0wc = np.ascontiguousarray(
            w0c.reshape(NCHUNK, 128, C).transpose(1, 0, 2).reshape(128, NCHUNK * C)
        )
        x1wc = np.ascontiguousarray(
            w1c.reshape(NCHUNK, 128, C).transpose(1, 0, 2).reshape(128, NCHUNK * C)
        )
        in_maps.append({"x0w": x0wc, "x1w": x1wc})
    return in_maps, (s0, s1)


def _run(in_maps, **kwargs):
    from concourse.bass_utils import run_bass_kernel_spmd

    if "nc" not in _cache:
        _cache["nc"] = _build_program()
    return run_bass_kernel_spmd(
        _cache["nc"], in_maps, core_ids=list(range(NCORES)), **kwargs
    )


def _finish(res, scales, x):
    s0, s1 = scales
    x = np.asarray(x)
    a = np.zeros((2, 128, C), dtype=np.float64)
    for r in res.results:
        a += r["aout"].astype(np.float64).reshape(128, 2, C).transpose(1, 0, 2)
    afull = a.reshape(2 * 128, C)
    vals = afull[x[:, 0].astype(np.int64), x[:, 1].astype(np.int64)]
    return (np.log(vals) - np.log(s0) - np.log(s1)).astype(np.float32)


def kernel(x, W, w_sum):
    in_maps, scales = _prep_inputs(x, W, w_sum)
    res = _run(in_maps)
    return _finish(res, scales, x)
